# Initial kernel scaffold
#
"""Self-contained Trainium2 Bass kernel for nn_BillboardAllocatorGNN.

Strategy:
  - 8 cores, data-parallel over batch: core c handles batch c//2 (pairs
    duplicate work in v1; host reads even cores' outputs).
  - Message passing: edges sorted by destination on host; nodes relabeled by
    in-degree rank so all 4 batches share ONE slot/chunk structure (max degree
    per rank across batches).  Gather h[src] rows via gpsimd.dma_gather from a
    DRAM node-major h table; segment-sum via PE matmuls msg_chunk^T @ S_chunk
    where S is a host-built block one-hot (batch independent), producing agg
    directly in feature-major layout.
  - Dense layers / heads all on PE+ACT+DVE; masked log-softmax uses
    gpsimd.partition_all_reduce for cross-partition max/sum.
"""

import hashlib
import os
import numpy as np
from contextlib import ExitStack

import concourse.bass as bass
import concourse.bass_isa as bass_isa
import concourse.mybir as mybir
from concourse import library_config
from concourse.bass_utils import run_bass_kernel_spmd

F32 = mybir.dt.float32
I16 = mybir.dt.int16
AF = mybir.ActivationFunctionType
ALU = mybir.AluOpType

P = 128
TILE_N = 256          # node-ranks per psum tile
NEG_INF = -1.0e9

# full-size problem constants
N_NODES = 20000
N_EDGES = 320000
BATCH = 4
NODE_F = 8
AD_F = 8
D = 128


# --------------------------------------------------------------------------
# host-side structure building
# --------------------------------------------------------------------------

def _prep_structure(edge_links, n_nodes):
    """edge_links: [B, 2, E] int32.  Returns common structure + per batch data.

    Node relabeling: per batch, rank = argsort(in-degree).  The slot structure
    is built from deg_max[r] = max over batches of the r-th smallest degree, so
    it is shared by all batches (each batch pads its extra slots with the zero
    row).
    """
    B = edge_links.shape[0]
    n_cols = ((n_nodes + TILE_N - 1) // TILE_N) * TILE_N
    zrow = n_cols                      # index of the all-zero row in h tables

    orders, rank_of, degs = [], [], []
    for b in range(B):
        dst = edge_links[b, 1]
        deg = np.bincount(dst, minlength=n_nodes)
        order = np.argsort(deg, kind="stable")        # rank -> node
        rk = np.empty(n_nodes, np.int64)
        rk[order] = np.arange(n_nodes)
        orders.append(order)
        rank_of.append(rk)
        degs.append(deg[order])                        # degree by rank
    deg_max = np.maximum.reduce(degs)                  # [n_nodes]
    slots_per_rank = np.maximum(deg_max, 1).astype(np.int64)
    slots_per_rank = np.concatenate(
        [slots_per_rank, np.ones(n_cols - n_nodes, np.int64)])
    assert slots_per_rank.max() <= P, "a node has more than 128 in-edges"

    # chunk building: chunks of 128 slots, whole ranks, within TILE_N tiles
    n_tiles = n_cols // TILE_N
    tiles = []          # per tile: list of (a_c, n_c, ch_slot_base)
    slot_start = np.zeros(n_cols, np.int64)   # global slot offset per rank
    slot_base = 0
    for t in range(n_tiles):
        chunks = []
        r = t * TILE_N
        end = (t + 1) * TILE_N
        while r < end:
            # start a new chunk at slot_base (128-aligned)
            a_c = r - t * TILE_N
            used = 0
            n_c = 0
            while r < end and used + slots_per_rank[r] <= P:
                slot_start[r] = slot_base + used
                used += slots_per_rank[r]
                n_c += 1
                r += 1
            chunks.append((a_c, n_c, slot_base))
            slot_base += P                    # chunk always occupies 128 slots
        tiles.append(chunks)
    n_slots = slot_base
    assert n_slots % P == 0

    # S matrix [128, n_cols]: S[slot_local, rank] = 1 for that rank's slots
    reps = slots_per_rank
    cols_rep = np.repeat(np.arange(n_cols), reps)
    starts_rep = np.repeat(slot_start, reps)
    within = np.arange(reps.sum()) - np.repeat(np.cumsum(reps) - reps, reps)
    rows_local = (starts_rep + within) % P
    S = np.zeros((P, n_cols), np.float32)
    S[rows_local, cols_rep] = 1.0

    # per-batch slot source indices (in rank space), padded with zrow
    slot_srcs = []
    for b in range(B):
        src = edge_links[b, 0]
        dst = edge_links[b, 1]
        dst_rank = rank_of[b][dst]
        src_rank = rank_of[b][src]
        order_e = np.argsort(dst_rank, kind="stable")
        ds = dst_rank[order_e]
        ss = src_rank[order_e]
        degb = degs[b]                                  # degree by rank
        starts_cum = np.concatenate([[0], np.cumsum(degb)])[:-1]
        within_e = np.arange(len(ds)) - starts_cum[ds]
        slot_arr = np.full(n_slots, zrow, np.int32)
        slot_arr[slot_start[ds] + within_e] = ss
        slot_srcs.append(slot_arr)

    # per-tile metadata
    tile_slot_off = []
    tile_nchunks = []
    for t in range(n_tiles):
        tile_slot_off.append(tiles[t][0][2])
        tile_nchunks.append(len(tiles[t]))
    tile_slot_off.append(n_slots)
    ch_max = max(tile_nchunks)

    return dict(
        n_cols=n_cols, zrow=zrow, n_tiles=n_tiles, tiles=tiles,
        n_slots=n_slots, S=S, slot_srcs=slot_srcs, orders=orders,
        rank_of=rank_of, tile_slot_off=tile_slot_off,
        tile_nchunks=tile_nchunks, ch_max=ch_max,
    )


def _wrap_idx(slot_arr):
    """[n_slots] -> [16, n_slots//16] int16 wrapped layout."""
    return np.ascontiguousarray(
        slot_arr.reshape(-1, 16).T.astype(np.int16))


# --------------------------------------------------------------------------
# device program builder
# --------------------------------------------------------------------------

class Ctr:
    """Counting semaphore helper; tracks expected value python-side."""

    def __init__(self, nc, name):
        self.sem = nc.alloc_semaphore(name)
        self.n = 0

    def inc(self, bass_ins, amt=1):
        bass_ins.then_inc(self.sem, amt)
        self.n += amt
        return bass_ins

    def wait(self, eng, val=None):
        v = self.n if val is None else val
        if v > 0:
            eng.wait_ge(self.sem, v)


def build_program(st, n_nodes, n_batch_nodes_f, ad_f, d, iters=1,
                  n_stat_cols=None, n_ad=4):
    """Build the bass program.  st: structure dict from _prep_structure."""
    n_cols = st["n_cols"]
    n_tiles = st["n_tiles"]
    n_slots = st["n_slots"]
    ch_max = st["ch_max"]
    zrow = st["zrow"]
    n_rows = n_cols + 1                 # h table rows (last = zero row)
    n_lg = n_cols // P                  # logits columns
    nf = n_batch_nodes_f                # node feature count (8)
    inv_count = 1.0 / float(n_stat_cols * P // nf)  # 1/(B*N) for stats

    nc = bass.Bass(trn_type="TRN2")

    # ---------------- I/O ----------------
    inp = {}

    def param(name, shape, dtype=F32):
        inp[name] = nc.declare_dram_parameter(name, list(shape), dtype,
                                              isOutput=False)
        return inp[name]

    stats_in = param("stats", [P, n_stat_cols])       # standardization layout
    nodesT_in = param("nodesT", [nf, n_cols])         # this batch, rank order
    adT_all = param("adT_all", [ad_f, n_ad])
    adT_own = param("adT_own", [ad_f, 1])
    idx_in = param("idx", [16, n_slots // 16], I16)
    S_in = param("S", [P, n_cols])
    maskF_in = param("maskF", [P, n_lg])
    maskOff_in = param("maskOff", [P, n_lg])
    W_in_in = param("W_in", [nf, d])
    b_in_in = param("b_in", [d, 1])
    W_m1_in = param("W_m1", [d, d])
    W_s1_in = param("W_s1", [d, d])
    b1_in = param("b1", [d, 1])
    W_m2_in = param("W_m2", [d, d])
    W_s2_in = param("W_s2", [d, d])
    b2_in = param("b2", [d, 1])
    W_ad_in = param("W_ad", [ad_f, d])
    b_ad_in = param("b_ad", [d, 1])
    W_pt_in = param("W_pt", [d, d])
    W_pb_in = param("W_pb", [d, d])
    b_p_in = param("b_p", [d, 1])
    w_out_in = param("w_out", [d, 1])
    W_vt_in = param("W_vt", [d, d])
    W_vb_in = param("W_vb", [d, d])
    b_v_in = param("b_v", [d, 1])
    w_vo_in = param("w_vo", [d, 1])
    ident_in = param("ident", [P, P])
    sel_in = param("SEL", [P, nf])                     # p -> p % nf one-hot
    one11_in = param("one11", [1, 1])

    out_lp = nc.declare_dram_parameter("out_lp", [P, n_lg], F32, isOutput=True)
    out_val = nc.declare_dram_parameter("out_val", [1, 1], F32, isOutput=True)

    # internal DRAM
    tables = [nc.dram_tensor(f"table{l}", [n_rows, d], F32) for l in range(3)]
    hTs = [nc.dram_tensor(f"hT{l}", [P, n_cols], F32) for l in range(2)]

    ctx_stack = ExitStack()
    sb = lambda shape, dtype=F32: ctx_stack.enter_context(
        nc.sbuf_tensor(shape, dtype))
    ps = lambda shape, dtype=F32: ctx_stack.enter_context(
        nc.psum_tensor(shape, dtype))

    # ---------------- SBUF steady state ----------------
    idx_sb = sb([16, n_slots // 16], I16)
    msg_sb = [sb([P, ch_max, d]) for _ in range(2)]
    S_sb = [sb([P, TILE_N]) for _ in range(2)]
    hTprev_sb = [sb([P, TILE_N]) for _ in range(2)]
    agg_sb = [sb([P, TILE_N]) for _ in range(2)]
    hTnew_sb = [sb([P, TILE_N]) for _ in range(2)]
    rows_sb = [sb([P, TILE_N // P, P]) for _ in range(2)]
    ph_sb = [sb([P, TILE_N]) for _ in range(2)]
    nodesT_sb = [sb([nf, TILE_N]) for _ in range(2)]
    lg_sb = sb([P, n_lg])
    lm_sb = sb([P, n_lg])           # masked logits
    e_sb = sb([P, n_lg])            # exp
    attn_sb = sb([P, n_lg])
    maskF_sb = sb([P, n_lg])
    maskOff_sb = sb([P, n_lg])
    red_sb = sb([P, 8])             # reduction scratch columns
    Wp_sb = sb([nf, d])             # folded W_in
    bp_sb = sb([d, 1])              # folded b_in
    W_sb = {}
    for nm, t in [("W_m1", W_m1_in), ("W_s1", W_s1_in), ("W_m2", W_m2_in),
                  ("W_s2", W_s2_in), ("W_pt", W_pt_in), ("W_pb", W_pb_in),
                  ("W_vt", W_vt_in), ("W_vb", W_vb_in)]:
        W_sb[nm] = sb([d, d])
    W_in_sb = sb([nf, d])
    W_ad_sb = sb([ad_f, d])
    vec_sb = {}
    for nm in ["b_in", "b1", "b2", "b_ad", "b_p", "w_out", "b_v", "w_vo"]:
        vec_sb[nm] = sb([d, 1])
    cb_sb = sb([d, 1])              # W_pb^T a + b_p
    cv_sb = sb([d, 1])              # W_vb^T a + b_v
    a_sb = sb([d, 1])               # ad embedding
    adT_all_sb = sb([ad_f, n_ad])
    adT_own_sb = sb([ad_f, 1])
    ident_sb = sb([P, P])
    sel_sb = sb([P, nf])
    one11_sb = sb([1, 1])
    mv_sb = sb([1, 16])             # [mu(nf) | E2(nf)] row
    var_sb = sb([1, 8])
    sd_sb = sb([1, 8])
    invf_sb = sb([1, 8])
    inv8_sb = sb([nf, 1])
    mu8_sb = sb([nf, 1])
    ms8_sb = sb([nf, 1])
    muad_sb = sb([ad_f, 1])
    e2ad_sb = sb([ad_f, 1])
    sdad_sb = sb([ad_f, 1])
    invad_sb = sb([ad_f, 1])
    adp_sb = sb([ad_f, 1])
    s1_sb = sb([P, 1])
    s2_sb = sb([P, 1])
    m_sb = sb([P, 1])               # local max
    M_sb = sb([P, 1])               # global max
    negM_sb = sb([P, 1])
    ssum_sb = sb([P, 1])
    Ssum_sb = sb([P, 1])
    lnS_sb = sb([P, 1])
    logZ_sb = sb([P, 1])
    recS_sb = sb([P, 1])
    ctx_sb = sb([d, 1])
    u_sb = sb([d, 1])
    val_sb = sb([1, 1])
    rowsc_sb = [sb([P, d]) for _ in range(2)]   # ctx row chunks

    # ---------------- PSUM ----------------
    ps_agg = [ps([P, TILE_N]) for _ in range(2)]
    ps_z = [ps([P, TILE_N]) for _ in range(2)]
    ps_tr = [ps([P, TILE_N // P, P]) for _ in range(2)]
    ps_lg = ps([P, max(n_lg, 16)])
    ps_misc = ps([P, 64])

    sync, pe, act, dve, gp = nc.sync, nc.tensor, nc.scalar, nc.vector, nc.gpsimd

    # counters
    c_w = Ctr(nc, "c_w")        # weight/const loads (dma, +16)
    c_g = Ctr(nc, "c_g")        # gathers
    c_s = Ctr(nc, "c_s")        # S tile loads
    c_hp = Ctr(nc, "c_hp")      # hT prev loads
    c_nd = Ctr(nc, "c_nd")      # nodesT loads
    c_pe = Ctr(nc, "c_pe")      # generic PE phase counter
    c_act = Ctr(nc, "c_act")    # generic ACT counter
    c_dve = Ctr(nc, "c_dve")    # generic DVE counter
    c_gp = Ctr(nc, "c_gp")      # gpsimd compute counter
    c_seg = Ctr(nc, "c_seg")    # PE done with segment mms of a tile
    c_cagg = Ctr(nc, "c_cagg")  # ACT copied agg tile
    c_z = Ctr(nc, "c_z")        # PE done dense z
    c_az = Ctr(nc, "c_az")      # ACT relu z -> hTnew
    c_tr = Ctr(nc, "c_tr")      # PE transposes done
    c_atr = Ctr(nc, "c_atr")    # ACT copied transpose
    c_wrow = Ctr(nc, "c_wrow")  # table row writes
    c_whT = Ctr(nc, "c_whT")    # hT writes
    c_ph = Ctr(nc, "c_ph")      # PE policy mm
    c_aph = Ctr(nc, "c_aph")    # ACT relu ph
    c_lg = Ctr(nc, "c_lg")      # PE logit mms
    c_rc = Ctr(nc, "c_rc")      # ctx row loads
    c_cx = Ctr(nc, "c_cx")      # ctx mms
    c_out = Ctr(nc, "c_out")    # output dmas
    ctrs = [c_w, c_g, c_s, c_hp, c_nd, c_pe, c_act, c_dve, c_gp, c_seg,
            c_cagg, c_z, c_az, c_tr, c_atr, c_wrow, c_whT, c_ph, c_aph,
            c_lg, c_rc, c_cx, c_out]

    gp.load_library(library_config.mlp)

    # ============ phase 0: load constants (outside loop) ============
    loads = [
        (idx_sb, idx_in), (maskF_sb, maskF_in), (maskOff_sb, maskOff_in),
        (W_in_sb, W_in_in), (W_ad_sb, W_ad_in), (ident_sb, ident_in),
        (sel_sb, sel_in), (one11_sb, one11_in),
        (adT_all_sb, adT_all), (adT_own_sb, adT_own),
        (W_sb["W_m1"], W_m1_in), (W_sb["W_s1"], W_s1_in),
        (W_sb["W_m2"], W_m2_in), (W_sb["W_s2"], W_s2_in),
        (W_sb["W_pt"], W_pt_in), (W_sb["W_pb"], W_pb_in),
        (W_sb["W_vt"], W_vt_in), (W_sb["W_vb"], W_vb_in),
        (vec_sb["b_in"], b_in_in), (vec_sb["b1"], b1_in),
        (vec_sb["b2"], b2_in), (vec_sb["b_ad"], b_ad_in),
        (vec_sb["b_p"], b_p_in), (vec_sb["w_out"], w_out_in),
        (vec_sb["b_v"], b_v_in), (vec_sb["w_vo"], w_vo_in),
    ]
    for dst, src in loads:
        c_w.inc(sync.dma_start(out=dst[:], in_=src[:]), 16)
    # zero row of tables
    zr_sb = sb([1, d])
    dve.memset(zr_sb[:], 0.0)
    dve_zr = nc.alloc_semaphore("zr")
    nc.vector.sem_inc(dve_zr, 1)
    sync.wait_ge(dve_zr, 1)
    for l in range(3):
        c_w.inc(sync.dma_start(out=tables[l][zrow:zrow + 1, :],
                               in_=zr_sb[:]), 16)

    with nc.Fori(0, iters):
        # ============ phase A: stats + folding ============
        with nc.sbuf_tensor([P, n_stat_cols], F32) as stats_sb, \
             nc.sbuf_tensor([P, n_stat_cols], F32) as sq_scratch:
            ld = sync.dma_start(out=stats_sb[:], in_=stats_in[:])
            c_w.inc(ld, 16)
            c_w.wait(dve)
            c_dve.inc(dve.reduce_sum(out=s1_sb[:], in_=stats_sb[:],
                                     axis=mybir.AxisListType.XYZW))
            c_w.wait(act)
            c_act.inc(act.activation(out=sq_scratch[:], in_=stats_sb[:],
                                     func=AF.Square, accum_out=s2_sb[:]))
            c_dve.wait(pe)      # s1 ready
            c_pe.inc(pe.matmul(out=ps_misc[:1, 0:nf], lhsT=s1_sb[:],
                               rhs=sel_sb[:], start=True, stop=True))
            c_act.wait(pe)      # s2 ready
            c_pe.inc(pe.matmul(out=ps_misc[:1, 8:8 + nf], lhsT=s2_sb[:],
                               rhs=sel_sb[:], start=True, stop=True))
            c_pe.wait(dve)
            c_dve.inc(dve.tensor_scalar_mul(out=mv_sb[:], in0=ps_misc[:1, 0:16],
                                            scalar1=inv_count))
            # var = E2 - mu^2
            c_dve.inc(dve.tensor_mul(out=var_sb[:], in0=mv_sb[:, 0:8],
                                     in1=mv_sb[:, 0:8]))
            c_dve.inc(dve.tensor_sub(out=var_sb[:], in0=mv_sb[:, 8:16],
                                     in1=var_sb[:]))
            c_dve.wait(act)
            c_act.inc(act.activation(out=sd_sb[:], in_=var_sb[:], func=AF.Sqrt))
            c_act.wait(dve)
            c_dve.inc(dve.tensor_scalar_add(out=sd_sb[:], in0=sd_sb[:],
                                            scalar1=1e-8))
            c_dve.inc(dve.reciprocal(out=invf_sb[:], in_=sd_sb[:]))
            c_dve.wait(pe)
            # transpose [1,nf] rows to [nf,1] columns via K=1 matmuls
            c_pe.inc(pe.matmul(out=ps_misc[0:nf, 16:17], lhsT=invf_sb[:, 0:nf],
                               rhs=one11_sb[:], start=True, stop=True))
            c_pe.inc(pe.matmul(out=ps_misc[0:nf, 17:18], lhsT=mv_sb[:, 0:nf],
                               rhs=one11_sb[:], start=True, stop=True))
            c_pe.wait(dve)
            c_dve.inc(dve.tensor_copy(out=inv8_sb[:], in_=ps_misc[0:nf, 16:17]))
            c_dve.inc(dve.tensor_copy(out=mu8_sb[:], in_=ps_misc[0:nf, 17:18]))
            # folded W_in and bias
            c_dve.inc(dve.tensor_scalar_mul(out=Wp_sb[:], in0=W_in_sb[:],
                                            scalar1=inv8_sb[:]))
            c_dve.inc(dve.tensor_mul(out=ms8_sb[:], in0=mu8_sb[:],
                                     in1=inv8_sb[:]))
            c_dve.wait(pe)
            c_pe.inc(pe.matmul(out=ps_misc[0:d, 18:19], lhsT=Wp_sb[:],
                               rhs=ms8_sb[:], start=True, stop=True))
            c_pe.wait(dve)
            c_dve.inc(dve.tensor_sub(out=bp_sb[:], in0=vec_sb["b_in"][:],
                                     in1=ps_misc[0:d, 18:19]))
            # ---- ad encoder ----
            c_dve.inc(dve.reduce_sum(out=muad_sb[:], in_=adT_all_sb[:],
                                     axis=mybir.AxisListType.XYZW))
            c_dve.inc(dve.tensor_scalar_mul(out=muad_sb[:], in0=muad_sb[:],
                                            scalar1=1.0 / n_ad))
            c_act.inc(act.activation(out=ps_misc[0:ad_f, 20:20 + n_ad],
                                     in_=adT_all_sb[:], func=AF.Square,
                                     accum_out=e2ad_sb[:]))
            c_act.wait(dve)
            c_dve.inc(dve.tensor_scalar_mul(out=e2ad_sb[:], in0=e2ad_sb[:],
                                            scalar1=1.0 / n_ad))
            c_dve.inc(dve.tensor_mul(out=sdad_sb[:], in0=muad_sb[:],
                                     in1=muad_sb[:]))
            c_dve.inc(dve.tensor_sub(out=sdad_sb[:], in0=e2ad_sb[:],
                                     in1=sdad_sb[:]))
            c_dve.wait(act)
            c_act.inc(act.activation(out=sdad_sb[:], in_=sdad_sb[:],
                                     func=AF.Sqrt))
            c_act.wait(dve)
            c_dve.inc(dve.tensor_scalar_add(out=sdad_sb[:], in0=sdad_sb[:],
                                            scalar1=1e-8))
            c_dve.inc(dve.reciprocal(out=invad_sb[:], in_=sdad_sb[:]))
            c_dve.inc(dve.tensor_scalar(out=adp_sb[:], in0=adT_own_sb[:],
                                        scalar1=muad_sb[:], scalar2=invad_sb[:],
                                        op0=ALU.subtract, op1=ALU.mult))
            c_dve.wait(pe)
            c_pe.inc(pe.matmul(out=ps_misc[0:d, 24:25], lhsT=W_ad_sb[:],
                               rhs=adp_sb[:], start=True, stop=True))
            c_pe.wait(act)
            c_act.inc(act.activation(out=a_sb[:], in_=ps_misc[0:d, 24:25],
                                     func=AF.Relu, bias=vec_sb["b_ad"][:]))
            c_act.wait(pe)
            c_pe.inc(pe.matmul(out=ps_misc[0:d, 26:27], lhsT=W_sb["W_pb"][:],
                               rhs=a_sb[:], start=True, stop=True))
            c_pe.inc(pe.matmul(out=ps_misc[0:d, 28:29], lhsT=W_sb["W_vb"][:],
                               rhs=a_sb[:], start=True, stop=True))
            c_pe.wait(dve)
            c_dve.inc(dve.tensor_add(out=cb_sb[:], in0=vec_sb["b_p"][:],
                                     in1=ps_misc[0:d, 26:27]))
            c_dve.inc(dve.tensor_add(out=cv_sb[:], in0=vec_sb["b_v"][:],
                                     in1=ps_misc[0:d, 28:29]))

            # ============ phase B: h0 ============
            for t in range(n_tiles):
                bi = t % 2
                cs = t * TILE_N
                # prefetch nodesT tile
                c_seg.wait(sync, c_seg.n)  # no-op ordering helper
                if t >= 2:
                    c_az.wait(sync, c_az.n - 1)
                c_nd.inc(sync.dma_start(out=nodesT_sb[bi][:],
                                        in_=nodesT_in[:, cs:cs + TILE_N]), 16)
                c_nd.wait(pe, 16 * (t + 1))
                if t == 0:
                    c_dve.wait(pe)      # folded weights ready
                if t >= 2:
                    c_az.wait(pe, c_az.n - 1)
                mm = pe.matmul(out=ps_z[bi][:], lhsT=Wp_sb[:],
                               rhs=nodesT_sb[bi][:], start=True, stop=True)
                c_z.inc(mm)
                c_z.wait(act)
                c_act2 = act.activation(out=hTnew_sb[bi][:], in_=ps_z[bi][:],
                                        func=AF.Relu, bias=bp_sb[:])
                c_az.inc(c_act2)
                # write hT0 + transpose to table0
                c_az.wait(sync)
                c_whT.inc(sync.dma_start(out=hTs[0][:, cs:cs + TILE_N],
                                         in_=hTnew_sb[bi][:]), 16)
                c_az.wait(pe)
                if t >= 2:
                    c_atr.wait(pe, c_atr.n - 1)
                for k in range(TILE_N // P):
                    tr = pe.transpose(out=ps_tr[bi][:, k, :],
                                      in_=hTnew_sb[bi][:, k * P:(k + 1) * P],
                                      identity=ident_sb[:])
                    if k == TILE_N // P - 1:
                        c_tr.inc(tr)
                c_tr.wait(act)
                if t >= 2:
                    c_wrow.wait(act, 16 * (t - 1))
                c_atr.inc(act.copy(out=rows_sb[bi][:], in_=ps_tr[bi][:]))
                c_atr.wait(sync)
                dst = tables[0][cs:cs + TILE_N, :].rearrange(
                    "(k p) f -> p k f", p=P)
                c_wrow.inc(sync.dma_start(out=dst, in_=rows_sb[bi][:]), 16)

        # ============ phase C: message passing layers ============
        for l in (1, 2):
            W_m = W_sb[f"W_m{l}"]
            W_s = W_sb[f"W_s{l}"]
            b_l = vec_sb[f"b{l}"]
            tbl_prev = tables[l - 1]
            hT_prev = hTs[l - 1]
            for t in range(n_tiles):
                bi = t % 2
                cs = t * TILE_N
                chunks = st["tiles"][t]
                so = st["tile_slot_off"][t]
                slots_t = len(chunks) * P
                # gather
                if c_seg.n >= 2:
                    c_seg.wait(gp, c_seg.n - 1)
                g = gp.dma_gather(
                    out_ap=msg_sb[bi][:, :len(chunks), :],
                    in_ap=tbl_prev[:],
                    idxs_ap=idx_sb[:, so // 16:(so + slots_t) // 16],
                    num_idxs=slots_t, num_idxs_reg=slots_t, elem_size=d)
                c_g.inc(g, 16)
                # S tile + hT prev tile
                if c_seg.n >= 2:
                    c_seg.wait(sync, c_seg.n - 1)
                c_s.inc(sync.dma_start(out=S_sb[bi][:],
                                       in_=S_in[:, cs:cs + TILE_N]), 16)
                if c_z.n >= 2:
                    c_z.wait(sync, c_z.n - 1)
                c_hp.inc(sync.dma_start(out=hTprev_sb[bi][:],
                                        in_=hT_prev[:, cs:cs + TILE_N]), 16)
                # PE segment matmuls
                c_g.wait(pe)
                c_s.wait(pe)
                if c_cagg.n >= 2:
                    c_cagg.wait(pe, c_cagg.n - 1)
                for ci, (a_c, n_c, _) in enumerate(chunks):
                    mm = pe.matmul(out=ps_agg[bi][:, a_c:a_c + n_c],
                                   lhsT=msg_sb[bi][:, ci, :],
                                   rhs=S_sb[bi][:, a_c:a_c + n_c],
                                   start=True, stop=True)
                c_seg.inc(mm)
                # ACT: copy agg
                c_seg.wait(act)
                c_cagg.inc(act.copy(out=agg_sb[bi][:], in_=ps_agg[bi][:]))
                # PE dense
                c_cagg.wait(pe)
                c_hp.wait(pe)
                if c_az.n >= 2:
                    c_az.wait(pe, c_az.n - 1)
                pe.matmul(out=ps_z[bi][:], lhsT=W_m[:], rhs=agg_sb[bi][:],
                          start=True, stop=False)
                c_z.inc(pe.matmul(out=ps_z[bi][:], lhsT=W_s[:],
                                  rhs=hTprev_sb[bi][:], start=False, stop=True))
                # ACT relu -> hTnew
                c_z.wait(act)
                if c_whT.n >= 2 and l == 1:
                    c_whT.wait(act, c_whT.n - 1)
                c_az.inc(act.activation(out=hTnew_sb[bi][:], in_=ps_z[bi][:],
                                        func=AF.Relu, bias=b_l[:]))
                if l == 1:
                    # write hT1 for next layer's W_s path
                    c_az.wait(sync)
                    c_whT.inc(sync.dma_start(out=hTs[1][:, cs:cs + TILE_N],
                                             in_=hTnew_sb[bi][:]), 16)
                # transposes -> table_l
                c_az.wait(pe)
                if c_atr.n >= 2:
                    c_atr.wait(pe, c_atr.n - 1)
                for k in range(TILE_N // P):
                    tr = pe.transpose(out=ps_tr[bi][:, k, :],
                                      in_=hTnew_sb[bi][:, k * P:(k + 1) * P],
                                      identity=ident_sb[:])
                    if k == TILE_N // P - 1:
                        c_tr.inc(tr)
                c_tr.wait(act)
                if c_wrow.n >= 2:
                    c_wrow.wait(act, c_wrow.n - 1)
                c_atr.inc(act.copy(out=rows_sb[bi][:], in_=ps_tr[bi][:]))
                c_atr.wait(sync)
                dst = tables[l][cs:cs + TILE_N, :].rearrange(
                    "(k p) f -> p k f", p=P)
                c_wrow.inc(sync.dma_start(out=dst, in_=rows_sb[bi][:]), 16)
                if l == 2:
                    # policy head inline: ph = relu(W_pt^T h + cb)
                    c_az.wait(pe)  # hTnew ready (already waited)
                    if c_aph.n >= 2:
                        c_aph.wait(pe, c_aph.n - 1)
                    c_ph.inc(pe.matmul(out=ps_z[bi][:], lhsT=W_sb["W_pt"][:],
                                       rhs=hTnew_sb[bi][:], start=True,
                                       stop=True))
                    c_ph.wait(act)
                    c_aph.inc(act.activation(out=ph_sb[bi][:], in_=ps_z[bi][:],
                                             func=AF.Relu, bias=cb_sb[:]))
                    c_aph.wait(pe)
                    for k in range(TILE_N // P):
                        mmlg = pe.matmul(
                            out=ps_lg[:, t * (TILE_N // P) + k:
                                      t * (TILE_N // P) + k + 1],
                            lhsT=ph_sb[bi][:, k * P:(k + 1) * P],
                            rhs=vec_sb["w_out"][:], start=True, stop=True)
                    c_lg.inc(mmlg)

        # ============ phase D: softmax + value ============
        c_lg.wait(act)
        c_act.inc(act.copy(out=lg_sb[:], in_=ps_lg[:, 0:n_lg]))
        c_act.wait(dve)
        c_dve.inc(dve.tensor_mul(out=lm_sb[:], in0=lg_sb[:], in1=maskF_sb[:]))
        c_dve.inc(dve.tensor_add(out=lm_sb[:], in0=lm_sb[:], in1=maskOff_sb[:]))
        c_dve.inc(dve.reduce_max(out=m_sb[:], in_=lm_sb[:],
                                 axis=mybir.AxisListType.XYZW))
        c_dve.wait(gp)
        c_gp.inc(gp.partition_all_reduce(out_ap=M_sb[:], in_ap=m_sb[:],
                                         channels=P,
                                         reduce_op=bass_isa.ReduceOp.max))
        c_gp.wait(dve)
        c_dve.inc(dve.tensor_scalar_mul(out=negM_sb[:], in0=M_sb[:],
                                        scalar1=-1.0))
        c_dve.wait(act)
        c_act.inc(act.activation(out=e_sb[:], in_=lm_sb[:], func=AF.Exp,
                                 bias=negM_sb[:]))
        c_act.wait(dve)
        c_dve.inc(dve.reduce_sum(out=ssum_sb[:], in_=e_sb[:],
                                 axis=mybir.AxisListType.XYZW))
        c_dve.wait(gp)
        c_gp.inc(gp.partition_all_reduce(out_ap=Ssum_sb[:], in_ap=ssum_sb[:],
                                         channels=P,
                                         reduce_op=bass_isa.ReduceOp.add))
        c_gp.wait(act)
        c_act.inc(act.activation(out=lnS_sb[:], in_=Ssum_sb[:], func=AF.Ln))
        c_act.wait(dve)
        c_dve.inc(dve.tensor_add(out=logZ_sb[:], in0=lnS_sb[:], in1=M_sb[:]))
        c_dve.inc(dve.tensor_scalar(out=lg_sb[:], in0=lm_sb[:],
                                    scalar1=logZ_sb[:], scalar2=None,
                                    op0=ALU.subtract))
        c_dve.inc(dve.reciprocal(out=recS_sb[:], in_=Ssum_sb[:]))
        c_dve.inc(dve.tensor_scalar_mul(out=attn_sb[:], in0=e_sb[:],
                                        scalar1=recS_sb[:]))
        c_dve.wait(sync)
        c_out.inc(sync.dma_start(out=out_lp[:], in_=lg_sb[:]), 16)

        # ctx: accumulate over row chunks of table2
        for j in range(n_lg):
            bj = j % 2
            if j >= 2:
                c_cx.wait(sync, c_cx.n - 1)
            else:
                c_wrow.wait(sync)   # table2 fully written
            c_rc.inc(sync.dma_start(out=rowsc_sb[bj][:],
                                    in_=tables[2][j * P:(j + 1) * P, :]), 16)
            c_rc.wait(pe)
            if j == 0:
                c_dve.wait(pe)      # attn ready
            mm = pe.matmul(out=ps_misc[0:d, 32:33], lhsT=rowsc_sb[bj][:],
                           rhs=attn_sb[:, j:j + 1], start=(j == 0),
                           stop=(j == n_lg - 1))
            c_cx.inc(mm)
        c_cx.wait(act)
        c_act.inc(act.copy(out=ctx_sb[:], in_=ps_misc[0:d, 32:33]))
        c_act.wait(pe)
        c_pe.inc(pe.matmul(out=ps_misc[0:d, 34:35], lhsT=W_sb["W_vt"][:],
                           rhs=ctx_sb[:], start=True, stop=True))
        c_pe.wait(act)
        c_act.inc(act.activation(out=u_sb[:], in_=ps_misc[0:d, 34:35],
                                 func=AF.Relu, bias=cv_sb[:]))
        c_act.wait(pe)
        c_pe.inc(pe.matmul(out=ps_misc[0:1, 36:37], lhsT=u_sb[:],
                           rhs=vec_sb["w_vo"][:], start=True, stop=True))
        c_pe.wait(act)
        c_act.inc(act.copy(out=val_sb[:], in_=ps_misc[0:1, 36:37]))
        c_act.wait(sync)
        c_out.inc(sync.dma_start(out=out_val[:], in_=val_sb[:]), 16)

        # ---- iteration epilogue: reset sems ----
        nc.all_engine_barrier()
        for ctr in ctrs:
            nc.sync.sem_clear(ctr.sem)
            ctr.n = 0
        nc.sync.sem_clear(dve_zr)
        nc.all_engine_barrier()
        c_w.n = 0

    ctx_stack.close()
    return nc


# --------------------------------------------------------------------------
# host-side input packing
# --------------------------------------------------------------------------

def _pack_inputs(inputs, st, b):
    """Build the per-core input map for batch b."""
    n_cols = st["n_cols"]
    n_lg = n_cols // P
    order = st["orders"][b]

    gn = np.asarray(inputs["graph_nodes"], np.float32)      # [B, N, 8]
    mask = np.asarray(inputs["mask"])                        # [B, N]
    ad = np.asarray(inputs["current_ad"], np.float32)        # [B, 8]
    B, N, nf = gn.shape

    flat = gn.reshape(-1, nf)                                # [B*N, nf]
    n_stat_cols = flat.shape[0] * nf // P
    stats = flat.reshape(16, -1, nf).transpose(0, 2, 1).reshape(P, n_stat_cols)

    nodesT = np.zeros((nf, n_cols), np.float32)
    nodesT[:, :N] = gn[b][order].T                           # rank order

    maskF = np.zeros((P, n_lg), np.float32)
    maskOff = np.full((P, n_lg), np.float32(NEG_INF))
    mrank = np.zeros(n_cols, bool)
    mrank[:N] = mask[b][order]
    mF = mrank.reshape(n_lg, P).T
    maskF[mF] = 1.0
    maskOff[mF] = 0.0

    sel = np.zeros((P, nf), np.float32)
    sel[np.arange(P), np.arange(P) % nf] = 1.0

    g = lambda k: np.ascontiguousarray(np.asarray(inputs[k], np.float32))
    col = lambda k: np.ascontiguousarray(
        np.asarray(inputs[k], np.float32).reshape(-1, 1))

    m = {
        "stats": np.ascontiguousarray(stats),
        "nodesT": np.ascontiguousarray(nodesT),
        "adT_all": np.ascontiguousarray(ad.T),
        "adT_own": np.ascontiguousarray(ad[b].reshape(-1, 1)),
        "idx": _wrap_idx(st["slot_srcs"][b]),
        "S": st["S"],
        "maskF": maskF, "maskOff": maskOff,
        "W_in": g("W_in"), "b_in": col("b_in"),
        "W_m1": g("W_m1"), "W_s1": g("W_s1"), "b1": col("b1"),
        "W_m2": g("W_m2"), "W_s2": g("W_s2"), "b2": col("b2"),
        "W_ad": g("W_ad"), "b_ad": col("b_ad"),
        "W_pt": g("W_p")[:D], "W_pb": g("W_p")[D:], "b_p": col("b_p"),
        "w_out": col("w_out"),
        "W_vt": g("W_v")[:D], "W_vb": g("W_v")[D:], "b_v": col("b_v"),
        "w_vo": col("w_vo"),
        "ident": np.eye(P, dtype=np.float32),
        "SEL": sel,
        "one11": np.ones((1, 1), np.float32),
    }
    return m


_CACHE = {}


def _get_program(edge_links, iters):
    key = (hashlib.md5(np.asarray(edge_links).tobytes()).hexdigest(), iters)
    if key not in _CACHE:
        st = _prep_structure(np.asarray(edge_links), N_NODES)
        flat_sz = BATCH * N_NODES * NODE_F
        nc = build_program(st, N_NODES, NODE_F, AD_F, D, iters=iters,
                           n_stat_cols=flat_sz // P, n_ad=BATCH)
        _CACHE[key] = (st, nc)
    return _CACHE[key]


def kernel(graph_nodes, graph_edge_links, mask, current_ad, **weights):
    inputs = dict(graph_nodes=graph_nodes, graph_edge_links=graph_edge_links,
                  mask=mask, current_ad=current_ad, **weights)
    edge_links = np.asarray(graph_edge_links)
    st, nc = _get_program(edge_links, iters=int(os.environ.get("K_ITERS", 1)))

    in_maps = []
    for c in range(8):
        b = c // 2
        in_maps.append(_pack_inputs(inputs, st, b))

    res = run_bass_kernel_spmd(nc, in_maps, core_ids=list(range(8)))

    n_cols = st["n_cols"]
    log_probs = np.zeros((BATCH, N_NODES), np.float32)
    value = np.zeros((BATCH,), np.float32)
    for b in range(BATCH):
        r = res.results[2 * b]
        lp_rank = r["out_lp"].T.reshape(-1)[:N_NODES]     # rank-major
        order = st["orders"][b]
        log_probs[b][order] = lp_rank
        value[b] = r["out_val"][0, 0]
    return log_probs, value


# revision 21
# speedup vs baseline: 2.5039x; 2.5039x over previous
"""Self-contained Trainium2 Bass kernel for nn_BillboardAllocatorGNN.

Strategy:
  - 8 cores, data-parallel over batch: core c handles batch c//2 (pairs
    duplicate work in v1; host reads even cores' outputs).
  - Message passing: edges sorted by destination on host; nodes relabeled by
    in-degree rank so all 4 batches share ONE slot/chunk structure (max degree
    per rank across batches).  Gather h[src] rows via one indirect_dma_start
    per 128-slot chunk (the only gather primitive that runs on this stack)
    from a DRAM node-major h table; segment-sum via PE matmuls
    msg_chunk^T @ S_chunk where S is a host-built block one-hot (batch
    independent), producing agg directly in feature-major layout.
  - Dense layers / heads all on PE+ACT+DVE; masked log-softmax does
    cross-partition max/sum via PE broadcast-transpose + DVE reduce +
    ones-matmul broadcast.
  - Timing (K_ITERS>1) uses python-unrolled iterations with monotonic
    semaphores and cross-iteration WAR guards (device-safe, unlike
    Fori+sem_clear which wedges the NCs).
  - Measured: ~1.29 s/iteration on HW, ~240 us per indirect-DMA gather
    (Q7 desc-gen bound) -- 5392 gathers dominate; all compute overlaps.
"""

import hashlib
import os
import numpy as np
from contextlib import ExitStack

import concourse.bass as bass
import concourse.bass_isa as bass_isa
import concourse.mybir as mybir
from concourse.bass_utils import run_bass_kernel_spmd

F32 = mybir.dt.float32
I16 = mybir.dt.int16
AF = mybir.ActivationFunctionType
ALU = mybir.AluOpType

P = 128
TILE_N = 256          # node-ranks per psum tile
NEG_INF = -1.0e9

# full-size problem constants
N_NODES = 20000
N_EDGES = 320000
BATCH = 4
NODE_F = 8
AD_F = 8
D = 128


# --------------------------------------------------------------------------
# host-side structure building
# --------------------------------------------------------------------------

def _prep_structure(edge_links, n_nodes):
    """edge_links: [B, 2, E] int32.  Returns common structure + per batch data.

    Node relabeling: per batch, rank = argsort(in-degree).  The slot structure
    is built from deg_max[r] = max over batches of the r-th smallest degree, so
    it is shared by all batches (each batch pads its extra slots with the zero
    row).
    """
    B = edge_links.shape[0]
    n_cols = ((n_nodes + TILE_N - 1) // TILE_N) * TILE_N
    zrow = n_cols                      # index of the all-zero row in h tables

    orders, rank_of, degs = [], [], []
    for b in range(B):
        dst = edge_links[b, 1]
        deg = np.bincount(dst, minlength=n_nodes)
        order = np.argsort(deg, kind="stable")        # rank -> node
        rk = np.empty(n_nodes, np.int64)
        rk[order] = np.arange(n_nodes)
        orders.append(order)
        rank_of.append(rk)
        degs.append(deg[order])                        # degree by rank
    deg_max = np.maximum.reduce(degs)                  # [n_nodes]
    # repacked: zero-degree ranks get no slots (psum memset covers them);
    # rank slots may split across 128-slot chunks (no fragmentation)
    slots_per_rank = deg_max.astype(np.int64)
    slots_per_rank = np.concatenate(
        [slots_per_rank, np.zeros(n_cols - n_nodes, np.int64)])
    assert slots_per_rank.max() <= P, "a node has more than 128 in-edges"

    n_tiles = n_cols // TILE_N
    tiles = []          # per tile: list of (a_c, n_c, ch_slot_base)
    slot_start = np.zeros(n_cols, np.int64)   # global slot offset per rank
    slot_base = 0
    for t in range(n_tiles):
        r0, r1 = t * TILE_N, (t + 1) * TILE_N
        ts_slots = 0
        for r in range(r0, r1):
            slot_start[r] = slot_base + ts_slots
            ts_slots += slots_per_rank[r]
        n_ch = max(1, -(-ts_slots // P))       # >=1 chunk per tile
        chunks = []
        for c in range(n_ch):
            s0 = slot_base + c * P
            s1 = s0 + P
            # ranks whose slot range intersects [s0, s1)
            cov = [r for r in range(r0, r1)
                   if slots_per_rank[r] > 0
                   and slot_start[r] < s1
                   and slot_start[r] + slots_per_rank[r] > s0]
            if cov:
                a_c = cov[0] - r0
                n_c = cov[-1] - cov[0] + 1
            else:
                a_c, n_c = 0, 1               # all-pad chunk (gathers zrow)
            chunks.append((a_c, n_c, s0))
        tiles.append(chunks)
        slot_base += n_ch * P                 # tile tail padded to 128
    n_slots = slot_base
    assert n_slots % P == 0

    # S stored as per-chunk blocks (a rank split across chunks appears as a
    # clipped column in EACH chunk's block, so no cross-chunk pollution).
    # chunk tuple becomes (a_c, n_c, s0, s_off) with s_off = col offset of
    # the chunk's block inside its tile's S stripe.
    s_tile_off = []          # per tile: col offset of its stripe in S
    s_tile_w = []            # per tile: stripe width
    S_cols = []
    s_total = 0
    for t in range(n_tiles):
        r0 = t * TILE_N
        s_tile_off.append(s_total)
        w_t = 0
        new_chunks = []
        for (a_c, n_c, s0) in tiles[t]:
            blk = np.zeros((P, n_c), np.float32)
            for j in range(n_c):
                r = r0 + a_c + j
                if slots_per_rank[r] == 0:
                    continue
                lo = max(slot_start[r], s0)
                hi = min(slot_start[r] + slots_per_rank[r], s0 + P)
                if lo < hi:
                    blk[lo - s0:hi - s0, j] = 1.0
            S_cols.append(blk)
            new_chunks.append((a_c, n_c, s0, w_t))
            w_t += n_c
        tiles[t] = new_chunks
        s_tile_w.append(w_t)
        s_total += w_t
    S = np.concatenate(S_cols, axis=1) if S_cols else np.zeros((P, 0), np.float32)
    assert S.shape[1] == s_total

    # per-batch slot source indices (in rank space), padded with zrow
    slot_srcs = []
    for b in range(B):
        src = edge_links[b, 0]
        dst = edge_links[b, 1]
        dst_rank = rank_of[b][dst]
        src_rank = rank_of[b][src]
        order_e = np.argsort(dst_rank, kind="stable")
        ds = dst_rank[order_e]
        ss = src_rank[order_e]
        degb = degs[b]                                  # degree by rank
        starts_cum = np.concatenate([[0], np.cumsum(degb)])[:-1]
        within_e = np.arange(len(ds)) - starts_cum[ds]
        slot_arr = np.full(n_slots, zrow, np.int32)
        slot_arr[slot_start[ds] + within_e] = ss
        slot_srcs.append(slot_arr)

    # per-tile metadata
    tile_slot_off = []
    tile_nchunks = []
    for t in range(n_tiles):
        tile_slot_off.append(tiles[t][0][2])
        tile_nchunks.append(len(tiles[t]))
    tile_slot_off.append(n_slots)
    ch_max = max(tile_nchunks)

    return dict(
        n_cols=n_cols, zrow=zrow, n_tiles=n_tiles, tiles=tiles,
        n_slots=n_slots, S=S, slot_srcs=slot_srcs, orders=orders,
        rank_of=rank_of, tile_slot_off=tile_slot_off,
        tile_nchunks=tile_nchunks, ch_max=ch_max,
        s_tile_off=s_tile_off, s_tile_w=s_tile_w, s_total=s_total,
    )


def _wrap_idx(slot_arr):
    """[n_slots] -> [128, n_chunks] int32: idx[s, c] = slot_src[c*128 + s]."""
    return np.ascontiguousarray(slot_arr.reshape(-1, 128).T.astype(np.int32))


# --------------------------------------------------------------------------
# device program builder
# --------------------------------------------------------------------------

class Ctr:
    """Counting semaphore helper; tracks expected value python-side."""

    def __init__(self, nc, name):
        self.sem = nc.alloc_semaphore(name)
        self.n = 0

    def inc(self, bass_ins, amt=1):
        bass_ins.then_inc(self.sem, amt)
        self.n += amt
        return bass_ins

    def wait(self, eng, val=None):
        v = self.n if val is None else val
        if v > 0:
            eng.wait_ge(self.sem, v)


def build_program(st, n_nodes, n_batch_nodes_f, ad_f, d, iters=1,
                  n_stat_cols=None, n_ad=4):
    """Build the bass program.  st: structure dict from _prep_structure."""
    n_cols = st["n_cols"]
    n_tiles = st["n_tiles"]
    n_slots = st["n_slots"]
    ch_max = st["ch_max"]
    zrow = st["zrow"]
    n_rows = n_cols + 1                 # h table rows (last = zero row)
    n_lg = n_cols // P                  # logits columns
    nf = n_batch_nodes_f                # node feature count (8)
    inv_count = 1.0 / float(n_stat_cols * P // nf)  # 1/(B*N) for stats

    nc = bass.Bass(trn_type="TRN2")

    # ---------------- I/O ----------------
    inp = {}

    def param(name, shape, dtype=F32):
        inp[name] = nc.declare_dram_parameter(name, list(shape), dtype,
                                              isOutput=False)
        return inp[name]

    stats_in = param("stats", [P, n_stat_cols])       # standardization layout
    nodesT_in = param("nodesT", [nf, n_cols])         # this batch, rank order
    adT_all = param("adT_all", [ad_f, n_ad])
    adT_own = param("adT_own", [ad_f, 1])
    idx_in = param("idx", [P, n_slots // P], mybir.dt.int32)
    s_total = st["s_total"]
    s_w_max = max(st["s_tile_w"])
    S_in = param("S", [P, s_total])
    maskF_in = param("maskF", [P, n_lg])
    maskOff_in = param("maskOff", [P, n_lg])
    W_in_in = param("W_in", [nf, d])
    b_in_in = param("b_in", [d, 1])
    W_m1_in = param("W_m1", [d, d])
    W_s1_in = param("W_s1", [d, d])
    b1_in = param("b1", [d, 1])
    W_m2_in = param("W_m2", [d, d])
    W_s2_in = param("W_s2", [d, d])
    b2_in = param("b2", [d, 1])
    W_ad_in = param("W_ad", [ad_f, d])
    b_ad_in = param("b_ad", [d, 1])
    W_pt_in = param("W_pt", [d, d])
    W_pb_in = param("W_pb", [d, d])
    b_p_in = param("b_p", [d, 1])
    w_out_in = param("w_out", [d, 1])
    W_vt_in = param("W_vt", [d, d])
    W_vb_in = param("W_vb", [d, d])
    b_v_in = param("b_v", [d, 1])
    w_vo_in = param("w_vo", [d, 1])
    ident_in = param("ident", [P, P])
    sel_in = param("SEL", [P, nf])                     # p -> p % nf one-hot
    one11_in = param("one11", [1, 1])
    ones128_in = param("ones128", [1, P])

    out_lp = nc.declare_dram_parameter("out_lp", [P, n_lg], F32, isOutput=True)
    out_val = nc.declare_dram_parameter("out_val", [1, 1], F32, isOutput=True)

    # internal DRAM
    tables = [nc.dram_tensor(f"table{l}", [n_rows, d], F32) for l in range(3)]
    hTs = [nc.dram_tensor(f"hT{l}", [P, n_cols], F32) for l in range(2)]

    ctx_stack = ExitStack()
    _nm = [0]

    def sb(shape, dtype=F32):
        _nm[0] += 1
        return ctx_stack.enter_context(
            nc.sbuf_tensor(f"sb{_nm[0]}", shape, dtype))

    def ps(shape, dtype=F32):
        _nm[0] += 1
        return ctx_stack.enter_context(
            nc.psum_tensor(f"ps{_nm[0]}", shape, dtype))

    # ---------------- SBUF steady state ----------------
    idx_sb = sb([P, n_slots // P], mybir.dt.int32)
    msg_sb = [sb([P, ch_max, d]) for _ in range(2)]
    S_sb = [sb([P, s_w_max]) for _ in range(2)]
    hTprev_sb = [sb([P, TILE_N]) for _ in range(2)]
    agg_sb = [sb([P, TILE_N]) for _ in range(2)]
    hTnew_sb = [sb([P, TILE_N]) for _ in range(2)]
    rows_sb = [sb([P, TILE_N // P, P]) for _ in range(2)]
    ph_sb = [sb([P, TILE_N]) for _ in range(2)]
    nodesT_sb = [sb([nf, TILE_N]) for _ in range(2)]
    lg_sb = sb([P, n_lg])
    lm_sb = sb([P, n_lg])           # masked logits
    e_sb = sb([P, n_lg])            # exp
    attn_sb = sb([P, n_lg])
    maskF_sb = sb([P, n_lg])
    maskOff_sb = sb([P, n_lg])
    red_sb = sb([P, 8])             # reduction scratch columns
    red1_sb = sb([P, 256])          # two-stage reduction scratch
    red2_sb = sb([P, 256])
    Wp_sb = sb([nf, d])             # folded W_in
    bp_sb = sb([d, 1])              # folded b_in
    W_sb = {}
    for nm, t in [("W_m1", W_m1_in), ("W_s1", W_s1_in), ("W_m2", W_m2_in),
                  ("W_s2", W_s2_in), ("W_pt", W_pt_in), ("W_pb", W_pb_in),
                  ("W_vt", W_vt_in), ("W_vb", W_vb_in)]:
        W_sb[nm] = sb([d, d])
    W_in_sb = sb([nf, d])
    W_ad_sb = sb([ad_f, d])
    vec_sb = {}
    for nm in ["b_in", "b1", "b2", "b_ad", "b_p", "w_out", "b_v", "w_vo"]:
        vec_sb[nm] = sb([d, 1])
    cb_sb = sb([d, 1])              # W_pb^T a + b_p
    cv_sb = sb([d, 1])              # W_vb^T a + b_v
    a_sb = sb([d, 1])               # ad embedding
    adT_all_sb = sb([ad_f, n_ad])
    adT_own_sb = sb([ad_f, 1])
    ident_sb = sb([P, P])
    sel_sb = sb([P, nf])
    one11_sb = sb([1, 1])
    ones128_sb = sb([1, P])
    sc1_sb = sb([1, 1])
    sc2_sb = sb([1, 1])
    mv_sb = sb([1, 16])             # [mu(nf) | E2(nf)] row
    var_sb = sb([1, 8])
    sd_sb = sb([1, 8])
    invf_sb = sb([1, 8])
    inv8_sb = sb([nf, 1])
    mu8_sb = sb([nf, 1])
    ms8_sb = sb([nf, 1])
    muad_sb = sb([ad_f, 1])
    e2ad_sb = sb([ad_f, 1])
    sdad_sb = sb([ad_f, 1])
    invad_sb = sb([ad_f, 1])
    adp_sb = sb([ad_f, 1])
    s1_sb = sb([P, 1])
    s2_sb = sb([P, 1])
    m_sb = sb([P, 1])               # local max
    M_sb = sb([P, 1])               # global max
    negM_sb = sb([P, 1])
    ssum_sb = sb([P, 1])
    Ssum_sb = sb([P, 1])
    lnS_sb = sb([P, 1])
    logZ_sb = sb([P, 1])
    recS_sb = sb([P, 1])
    ctx_sb = sb([d, 1])
    u_sb = sb([d, 1])
    val_sb = sb([1, 1])
    rowsc_sb = [sb([P, d]) for _ in range(2)]   # ctx row chunks

    # ---------------- PSUM ----------------
    ps_agg = [ps([P, TILE_N]) for _ in range(2)]
    ps_z = [ps([P, TILE_N]) for _ in range(2)]
    ps_tr = [ps([P, TILE_N // P, P]) for _ in range(2)]
    ps_lg = ps([P, max(n_lg, 16)])
    ps_misc = ps([P, 64])

    sync, pe, act, dve, gp = nc.sync, nc.tensor, nc.scalar, nc.vector, nc.gpsimd

    # counters
    c_w = Ctr(nc, "c_w")        # weight/const loads (dma, +16)
    c_g = [Ctr(nc, "c_g0"), Ctr(nc, "c_g1")]      # gathers (parity)
    c_s = [Ctr(nc, "c_s0"), Ctr(nc, "c_s1")]      # S tile loads
    c_hp = [Ctr(nc, "c_hp0"), Ctr(nc, "c_hp1")]   # hT prev loads
    c_nd = [Ctr(nc, "c_nd0"), Ctr(nc, "c_nd1")]   # nodesT loads
    c_pe = Ctr(nc, "c_pe")      # generic PE phase counter
    c_act = Ctr(nc, "c_act")    # generic ACT counter
    c_dve = Ctr(nc, "c_dve")    # generic DVE counter
    c_gp = Ctr(nc, "c_gp")      # gpsimd compute counter
    c_ps = Ctr(nc, "c_ps")      # DVE psum-agg memset counter
    c_seg = Ctr(nc, "c_seg")    # PE done with segment mms of a tile
    c_cagg = Ctr(nc, "c_cagg")  # ACT copied agg tile
    c_z = Ctr(nc, "c_z")        # PE done dense z
    c_az = Ctr(nc, "c_az")      # ACT relu z -> hTnew
    c_tr = Ctr(nc, "c_tr")      # PE transposes done
    c_atr = Ctr(nc, "c_atr")    # ACT copied transpose
    c_wrow = [Ctr(nc, "c_wrow0"), Ctr(nc, "c_wrow1")]  # table row writes
    c_whT = [Ctr(nc, "c_whT0"), Ctr(nc, "c_whT1")]  # hT writes
    c_ph = Ctr(nc, "c_ph")      # PE policy mm
    c_aph = Ctr(nc, "c_aph")    # ACT relu ph
    c_lg = Ctr(nc, "c_lg")      # PE logit mms
    c_rc = [Ctr(nc, "c_rc0"), Ctr(nc, "c_rc1")]   # ctx row loads
    c_cx = Ctr(nc, "c_cx")      # ctx mms
    c_out = Ctr(nc, "c_out")    # output dmas
    ctrs = [c_w, *c_g, *c_s, *c_hp, *c_nd, c_pe, c_act, c_dve, c_gp, c_seg,
            c_cagg, c_z, c_az, c_tr, c_atr, *c_wrow, *c_whT, c_ph, c_aph,
            c_lg, *c_rc, c_cx, c_out]


    # ============ phase 0: load constants (outside loop) ============
    loads = [
        (idx_sb, idx_in), (maskF_sb, maskF_in), (maskOff_sb, maskOff_in),
        (W_in_sb, W_in_in), (W_ad_sb, W_ad_in), (ident_sb, ident_in),
        (sel_sb, sel_in), (one11_sb, one11_in), (ones128_sb, ones128_in),
        (adT_all_sb, adT_all), (adT_own_sb, adT_own),
        (W_sb["W_m1"], W_m1_in), (W_sb["W_s1"], W_s1_in),
        (W_sb["W_m2"], W_m2_in), (W_sb["W_s2"], W_s2_in),
        (W_sb["W_pt"], W_pt_in), (W_sb["W_pb"], W_pb_in),
        (W_sb["W_vt"], W_vt_in), (W_sb["W_vb"], W_vb_in),
        (vec_sb["b_in"], b_in_in), (vec_sb["b1"], b1_in),
        (vec_sb["b2"], b2_in), (vec_sb["b_ad"], b_ad_in),
        (vec_sb["b_p"], b_p_in), (vec_sb["w_out"], w_out_in),
        (vec_sb["b_v"], b_v_in), (vec_sb["w_vo"], w_vo_in),
    ]
    for dst, src in loads:
        c_w.inc(sync.dma_start(out=dst[:], in_=src[:]), 16)
    zeroS_sb = sb([P, TILE_N])
    dve.memset(zeroS_sb[:], 0.0)
    # zero row of tables
    zr_sb = sb([1, d])
    dve_zr = nc.alloc_semaphore("zr")
    dve.memset(zr_sb[:], 0.0).then_inc(dve_zr, 1)
    sync.wait_ge(dve_zr, 1)
    for l in range(3):
        c_w.inc(sync.dma_start(out=tables[l][zrow:zrow + 1, :],
                               in_=zr_sb[:]), 16)

    for _it in range(iters):
        # ============ phase A: stats + folding ============
        with nc.sbuf_tensor([P, n_stat_cols], F32) as stats_sb, \
             nc.sbuf_tensor([P, n_stat_cols], F32) as sq_scratch:
            ld = sync.dma_start(out=stats_sb[:], in_=stats_in[:])
            c_w.inc(ld, 16)
            c_w.wait(dve)
            c_dve.inc(dve.reduce_sum(out=s1_sb[:], in_=stats_sb[:],
                                     axis=mybir.AxisListType.X))
            c_w.wait(act)
            c_act.inc(act.activation(out=sq_scratch[:], in_=stats_sb[:],
                                     func=AF.Square, accum_out=s2_sb[:]))
            if c_act.n > 0:
                c_act.wait(pe)  # prev-iter ACT reads of ps_misc done
            c_dve.wait(pe)      # s1 ready
            c_pe.inc(pe.matmul(out=ps_misc[:1, 0:nf], lhsT=s1_sb[:],
                               rhs=sel_sb[:], start=True, stop=True))
            c_dve.wait(pe)      # s2 ready
            c_pe.inc(pe.matmul(out=ps_misc[:1, 8:8 + nf], lhsT=s2_sb[:],
                               rhs=sel_sb[:], start=True, stop=True))
            c_pe.wait(dve)
            c_dve.inc(dve.tensor_scalar_mul(out=mv_sb[:], in0=ps_misc[:1, 0:16],
                                            scalar1=inv_count))
            # var = E2 - mu^2
            c_dve.inc(dve.tensor_mul(out=var_sb[:], in0=mv_sb[:, 0:8],
                                     in1=mv_sb[:, 0:8]))
            c_dve.inc(dve.tensor_sub(out=var_sb[:], in0=mv_sb[:, 8:16],
                                     in1=var_sb[:]))
            c_dve.wait(act)
            c_act.inc(act.activation(out=sd_sb[:], in_=var_sb[:], func=AF.Sqrt))
            c_act.wait(dve)
            c_dve.inc(dve.tensor_scalar_add(out=sd_sb[:], in0=sd_sb[:],
                                            scalar1=1e-8))
            c_dve.inc(dve.reciprocal(out=invf_sb[:], in_=sd_sb[:]))
            c_dve.wait(pe)
            # transpose [1,nf] rows to [nf,1] columns via K=1 matmuls
            c_pe.inc(pe.matmul(out=ps_misc[0:nf, 16:17], lhsT=invf_sb[:, 0:nf],
                               rhs=one11_sb[:], start=True, stop=True))
            c_pe.inc(pe.matmul(out=ps_misc[0:nf, 17:18], lhsT=mv_sb[:, 0:nf],
                               rhs=one11_sb[:], start=True, stop=True))
            c_pe.wait(dve)
            c_dve.inc(dve.tensor_copy(out=inv8_sb[:], in_=ps_misc[0:nf, 16:17]))
            c_dve.inc(dve.tensor_copy(out=mu8_sb[:], in_=ps_misc[0:nf, 17:18]))
            # folded W_in and bias
            c_dve.inc(dve.tensor_scalar_mul(out=Wp_sb[:], in0=W_in_sb[:],
                                            scalar1=inv8_sb[:]))
            c_dve.inc(dve.tensor_mul(out=ms8_sb[:], in0=mu8_sb[:],
                                     in1=inv8_sb[:]))
            c_dve.wait(pe)
            c_pe.inc(pe.matmul(out=ps_misc[0:d, 18:19], lhsT=Wp_sb[:],
                               rhs=ms8_sb[:], start=True, stop=True))
            c_pe.wait(dve)
            c_dve.inc(dve.tensor_sub(out=bp_sb[:], in0=vec_sb["b_in"][:],
                                     in1=ps_misc[0:d, 18:19]))
            # ---- ad encoder ----
            c_dve.inc(dve.reduce_sum(out=muad_sb[:], in_=adT_all_sb[:],
                                     axis=mybir.AxisListType.X))
            c_dve.inc(dve.tensor_scalar_mul(out=muad_sb[:], in0=muad_sb[:],
                                            scalar1=1.0 / n_ad))
            c_act.inc(act.activation(out=ps_misc[0:ad_f, 20:20 + n_ad],
                                     in_=adT_all_sb[:], func=AF.Square,
                                     accum_out=e2ad_sb[:]))
            c_act.wait(dve)
            c_dve.inc(dve.tensor_scalar_mul(out=e2ad_sb[:], in0=e2ad_sb[:],
                                            scalar1=1.0 / n_ad))
            c_dve.inc(dve.tensor_mul(out=sdad_sb[:], in0=muad_sb[:],
                                     in1=muad_sb[:]))
            c_dve.inc(dve.tensor_sub(out=sdad_sb[:], in0=e2ad_sb[:],
                                     in1=sdad_sb[:]))
            c_dve.wait(act)
            c_act.inc(act.activation(out=sdad_sb[:], in_=sdad_sb[:],
                                     func=AF.Sqrt))
            c_act.wait(dve)
            c_dve.inc(dve.tensor_scalar_add(out=sdad_sb[:], in0=sdad_sb[:],
                                            scalar1=1e-8))
            c_dve.inc(dve.reciprocal(out=invad_sb[:], in_=sdad_sb[:]))
            c_dve.inc(dve.tensor_scalar(out=adp_sb[:], in0=adT_own_sb[:],
                                        scalar1=muad_sb[:], scalar2=invad_sb[:],
                                        op0=ALU.subtract, op1=ALU.mult))
            c_dve.wait(pe)
            c_pe.inc(pe.matmul(out=ps_misc[0:d, 24:25], lhsT=W_ad_sb[:],
                               rhs=adp_sb[:], start=True, stop=True))
            c_pe.wait(act)
            c_act.inc(act.activation(out=a_sb[:], in_=ps_misc[0:d, 24:25],
                                     func=AF.Relu, bias=vec_sb["b_ad"][:]))
            c_act.wait(pe)
            c_pe.inc(pe.matmul(out=ps_misc[0:d, 26:27], lhsT=W_sb["W_pb"][:],
                               rhs=a_sb[:], start=True, stop=True))
            c_pe.inc(pe.matmul(out=ps_misc[0:d, 28:29], lhsT=W_sb["W_vb"][:],
                               rhs=a_sb[:], start=True, stop=True))
            c_pe.wait(dve)
            c_dve.inc(dve.tensor_add(out=cb_sb[:], in0=vec_sb["b_p"][:],
                                     in1=ps_misc[0:d, 26:27]))
            c_dve.inc(dve.tensor_add(out=cv_sb[:], in0=vec_sb["b_v"][:],
                                     in1=ps_misc[0:d, 28:29]))

            # ============ phase B: h0 ============
            for t in range(n_tiles):
                bi = t % 2
                cs = t * TILE_N
                if t == 0 and c_g[0].n > 0:
                    # prev-iteration consumers of table0/hT0 must finish
                    for cc in (c_g[0], c_g[1], c_hp[0], c_hp[1]):
                        cc.wait(sync)
                # prefetch nodesT tile
                if t >= 2:
                    c_az.wait(sync, c_az.n - 1)
                c_nd[bi].inc(sync.dma_start(out=nodesT_sb[bi][:],
                                        in_=nodesT_in[:, cs:cs + TILE_N]), 16)
                c_nd[bi].wait(pe)
                if t == 0:
                    c_dve.wait(pe)      # folded weights ready
                if t >= 2:
                    c_az.wait(pe, c_az.n - 1)
                mm = pe.matmul(out=ps_z[bi][:], lhsT=Wp_sb[:],
                               rhs=nodesT_sb[bi][:], start=True, stop=True)
                c_z.inc(mm)
                c_z.wait(act)
                if t >= 2:
                    c_whT[bi].wait(act)
                c_act2 = act.activation(out=hTnew_sb[bi][:], in_=ps_z[bi][:],
                                        func=AF.Relu, bias=bp_sb[:])
                c_az.inc(c_act2)
                # write hT0 + transpose to table0
                c_az.wait(sync)
                c_whT[bi].inc(sync.dma_start(out=hTs[0][:, cs:cs + TILE_N],
                                             in_=hTnew_sb[bi][:]), 16)
                c_az.wait(pe)
                if t >= 2:
                    c_atr.wait(pe, c_atr.n - 1)
                for k in range(TILE_N // P):
                    tr = pe.transpose(out=ps_tr[bi][:, k, :],
                                      in_=hTnew_sb[bi][:, k * P:(k + 1) * P],
                                      identity=ident_sb[:])
                    if k == TILE_N // P - 1:
                        c_tr.inc(tr)
                c_tr.wait(act)
                if t >= 2:
                    c_wrow[bi].wait(act)
                c_atr.inc(act.copy(out=rows_sb[bi][:], in_=ps_tr[bi][:]))
                c_atr.wait(sync)
                dst = tables[0][cs:cs + TILE_N, :].rearrange(
                    "(k p) f -> p k f", p=P)
                c_wrow[bi].inc(sync.dma_start(out=dst, in_=rows_sb[bi][:]), 16)

        # ============ phase C: message passing layers ============
        for l in (1, 2):
            W_m = W_sb[f"W_m{l}"]
            W_s = W_sb[f"W_s{l}"]
            b_l = vec_sb[f"b{l}"]
            tbl_prev = tables[l - 1]
            hT_prev = hTs[l - 1]
            for t in range(n_tiles):
                bi = t % 2
                cs = t * TILE_N
                chunks = st["tiles"][t]
                so = st["tile_slot_off"][t]
                slots_t = len(chunks) * P
                if t == 0:
                    # table_{l-1} fully written (incl. zero row) before gathers
                    c_w.wait(gp)
                    c_wrow[0].wait(gp)
                    c_wrow[1].wait(gp)
                    # hT_{l-1} fully written before hp loads (DMA RAW on sync)
                    c_whT[0].wait(sync)
                    c_whT[1].wait(sync)
                    if c_g[0].n > 0 and l == 1:
                        # prev-iter l=2 gathers read table1; hp loads read hT1
                        c_g[0].wait(sync)
                        c_g[1].wait(sync)
                        c_hp[0].wait(sync)
                        c_hp[1].wait(sync)
                    if l == 2 and c_rc[0].n > 0:
                        # prev-iter ctx row loads read table2
                        c_rc[0].wait(sync)
                        c_rc[1].wait(sync)
                # gather: one indirect DMA per 128-slot chunk
                if c_seg.n >= 2:
                    c_seg.wait(gp, c_seg.n - 1)
                ch0 = so // P
                for ci in range(len(chunks)):
                    g = gp.indirect_dma_start(
                        out=msg_sb[bi][:, ci, :], out_offset=None,
                        in_=tbl_prev[:],
                        in_offset=bass.IndirectOffsetOnAxis(
                            ap=idx_sb[:, ch0 + ci:ch0 + ci + 1], axis=0))
                    c_g[bi].inc(g, 16)
                # S tile + hT prev tile
                if c_seg.n >= 2:
                    c_seg.wait(sync, c_seg.n - 1)
                s_off = st["s_tile_off"][t]
                s_w = st["s_tile_w"][t]
                c_s[bi].inc(sync.dma_start(
                    out=S_sb[bi][:, :s_w],
                    in_=S_in[:, s_off:s_off + s_w]), 16)
                if c_z.n >= 2:
                    c_z.wait(sync, c_z.n - 1)
                c_hp[bi].inc(sync.dma_start(out=hTprev_sb[bi][:],
                                             in_=hT_prev[:, cs:cs + TILE_N]), 16)
                # PE segment matmuls: full-width start=True zeroing matmul
                # (sets PSUM accumulation bits), then accumulating chunks
                c_g[bi].wait(pe)
                c_s[bi].wait(pe)
                if c_cagg.n >= 2:
                    c_cagg.wait(pe, c_cagg.n - 1)
                pe.matmul(out=ps_agg[bi][:], lhsT=msg_sb[bi][:, 0, :],
                          rhs=zeroS_sb[:], start=True, stop=False,
                          skip_group_check=True)
                for ci, (a_c, n_c, _, so_c) in enumerate(chunks):
                    mm = pe.matmul(out=ps_agg[bi][:, a_c:a_c + n_c],
                                   lhsT=msg_sb[bi][:, ci, :],
                                   rhs=S_sb[bi][:, so_c:so_c + n_c],
                                   start=False, stop=True,
                                   skip_group_check=True)
                c_seg.inc(mm)
                # ACT: copy agg
                c_seg.wait(act)
                c_cagg.inc(act.copy(out=agg_sb[bi][:], in_=ps_agg[bi][:]))
                # PE dense
                c_cagg.wait(pe)
                c_hp[bi].wait(pe)
                if c_az.n >= 2:
                    c_az.wait(pe, c_az.n - 1)
                if l == 2 and c_aph.n >= 2:
                    c_aph.wait(pe, c_aph.n - 1)  # ps_z also held by relu_ph
                pe.matmul(out=ps_z[bi][:], lhsT=W_m[:], rhs=agg_sb[bi][:],
                          start=True, stop=False)
                c_z.inc(pe.matmul(out=ps_z[bi][:], lhsT=W_s[:],
                                  rhs=hTprev_sb[bi][:], start=False, stop=True))
                # ACT relu -> hTnew (hTnew buffer may still be read by a
                # pending hT write dma from 2 tiles ago / previous layer)
                c_z.wait(act)
                if l == 1:
                    c_whT[bi].wait(act)
                elif t < 2:
                    c_whT[0].wait(act)
                    c_whT[1].wait(act)
                c_az.inc(act.activation(out=hTnew_sb[bi][:], in_=ps_z[bi][:],
                                        func=AF.Relu, bias=b_l[:]))
                if l == 1:
                    # write hT1 for next layer's W_s path
                    c_az.wait(sync)
                    c_whT[bi].inc(sync.dma_start(out=hTs[1][:, cs:cs + TILE_N],
                                                 in_=hTnew_sb[bi][:]), 16)
                # transposes -> table_l
                c_az.wait(pe)
                if c_atr.n >= 2:
                    c_atr.wait(pe, c_atr.n - 1)
                for k in range(TILE_N // P):
                    tr = pe.transpose(out=ps_tr[bi][:, k, :],
                                      in_=hTnew_sb[bi][:, k * P:(k + 1) * P],
                                      identity=ident_sb[:])
                    if k == TILE_N // P - 1:
                        c_tr.inc(tr)
                c_tr.wait(act)
                c_wrow[bi].wait(act)
                c_atr.inc(act.copy(out=rows_sb[bi][:], in_=ps_tr[bi][:]))
                c_atr.wait(sync)
                dst = tables[l][cs:cs + TILE_N, :].rearrange(
                    "(k p) f -> p k f", p=P)
                c_wrow[bi].inc(sync.dma_start(out=dst, in_=rows_sb[bi][:]), 16)
                if l == 2:
                    # policy head inline: ph = relu(W_pt^T h + cb)
                    c_az.wait(pe)  # hTnew ready (already waited)
                    if c_aph.n >= 2:
                        c_aph.wait(pe, c_aph.n - 1)
                    c_ph.inc(pe.matmul(out=ps_z[bi][:], lhsT=W_sb["W_pt"][:],
                                       rhs=hTnew_sb[bi][:], start=True,
                                       stop=True))
                    c_ph.wait(act)
                    if c_lg.n >= 2:
                        c_lg.wait(act, c_lg.n - 1)  # ph_sb buffer reuse
                    c_aph.inc(act.activation(out=ph_sb[bi][:], in_=ps_z[bi][:],
                                             func=AF.Relu, bias=cb_sb[:]))
                    c_aph.wait(pe)
                    for k in range(TILE_N // P):
                        mmlg = pe.matmul(
                            out=ps_lg[:, t * (TILE_N // P) + k:
                                      t * (TILE_N // P) + k + 1],
                            lhsT=ph_sb[bi][:, k * P:(k + 1) * P],
                            rhs=vec_sb["w_out"][:], start=True, stop=True)
                    c_lg.inc(mmlg)

        # ============ phase D: softmax + value ============
        c_lg.wait(act)
        c_act.inc(act.copy(out=lg_sb[:], in_=ps_lg[:, 0:n_lg]))
        c_act.wait(dve)
        c_dve.inc(dve.tensor_mul(out=lm_sb[:], in0=lg_sb[:], in1=maskF_sb[:]))
        c_dve.inc(dve.tensor_add(out=lm_sb[:], in0=lm_sb[:], in1=maskOff_sb[:]))
        c_dve.inc(dve.reduce_max(out=m_sb[:], in_=lm_sb[:],
                                 axis=mybir.AxisListType.X))
        c_dve.wait(gp)
        c_gp.inc(gp.partition_all_reduce(out_ap=M_sb[:], in_ap=m_sb[:],
                                         channels=P,
                                         reduce_op=bass_isa.ReduceOp.max))
        c_gp.wait(dve)
        c_dve.inc(dve.tensor_scalar_mul(out=negM_sb[:], in0=M_sb[:],
                                        scalar1=-1.0))
        c_dve.wait(act)
        c_act.inc(act.activation(out=e_sb[:], in_=lm_sb[:], func=AF.Exp,
                                 bias=negM_sb[:]))
        c_act.wait(dve)
        c_dve.inc(dve.reduce_sum(out=ssum_sb[:], in_=e_sb[:],
                                 axis=mybir.AxisListType.X))
        c_dve.wait(gp)
        c_gp.inc(gp.partition_all_reduce(out_ap=Ssum_sb[:], in_ap=ssum_sb[:],
                                         channels=P,
                                         reduce_op=bass_isa.ReduceOp.add))
        c_gp.wait(act)
        c_act.inc(act.activation(out=lnS_sb[:], in_=Ssum_sb[:], func=AF.Ln))
        c_act.wait(dve)
        c_dve.inc(dve.tensor_add(out=logZ_sb[:], in0=lnS_sb[:], in1=M_sb[:]))
        c_dve.inc(dve.tensor_scalar(out=lg_sb[:], in0=lm_sb[:],
                                    scalar1=logZ_sb[:], scalar2=None,
                                    op0=ALU.subtract))
        c_dve.inc(dve.reciprocal(out=recS_sb[:], in_=Ssum_sb[:]))
        c_dve.inc(dve.tensor_scalar_mul(out=attn_sb[:], in0=e_sb[:],
                                        scalar1=recS_sb[:]))
        c_dve.wait(sync)
        c_out.inc(sync.dma_start(out=out_lp[:], in_=lg_sb[:]), 16)

        # ctx: accumulate over row chunks of table2
        for j in range(n_lg):
            bj = j % 2
            if j >= 2:
                c_cx.wait(sync, c_cx.n - 1)
            else:
                c_wrow[0].wait(sync)   # table2 fully written
                c_wrow[1].wait(sync)
            c_rc[bj].inc(sync.dma_start(out=rowsc_sb[bj][:],
                                        in_=tables[2][j * P:(j + 1) * P, :]), 16)
            c_rc[bj].wait(pe)
            if j == 0:
                c_dve.wait(pe)      # attn ready
            mm = pe.matmul(out=ps_misc[0:d, 32:33], lhsT=rowsc_sb[bj][:],
                           rhs=attn_sb[:, j:j + 1], start=(j == 0),
                           stop=(j == n_lg - 1))
            c_cx.inc(mm)
        c_cx.wait(act)
        c_act.inc(act.copy(out=ctx_sb[:], in_=ps_misc[0:d, 32:33]))
        c_act.wait(pe)
        c_pe.inc(pe.matmul(out=ps_misc[0:d, 34:35], lhsT=W_sb["W_vt"][:],
                           rhs=ctx_sb[:], start=True, stop=True))
        c_pe.wait(act)
        c_act.inc(act.activation(out=u_sb[:], in_=ps_misc[0:d, 34:35],
                                 func=AF.Relu, bias=cv_sb[:]))
        c_act.wait(pe)
        c_pe.inc(pe.matmul(out=ps_misc[0:1, 36:37], lhsT=u_sb[:],
                           rhs=vec_sb["w_vo"][:], start=True, stop=True))
        c_pe.wait(act)
        c_act.inc(act.copy(out=val_sb[:], in_=ps_misc[0:1, 36:37]))
        c_act.wait(sync)
        c_out.wait(sync)
        c_out.inc(sync.dma_start(out=out_val[:], in_=val_sb[:]), 16)

        # monotonic semaphores: no per-iteration reset needed

    ctx_stack.close()
    return nc


# --------------------------------------------------------------------------
# host-side input packing
# --------------------------------------------------------------------------

def _pack_inputs(inputs, st, b):
    """Build the per-core input map for batch b."""
    n_cols = st["n_cols"]
    n_lg = n_cols // P
    order = st["orders"][b]

    gn = np.asarray(inputs["graph_nodes"], np.float32)      # [B, N, 8]
    mask = np.asarray(inputs["mask"])                        # [B, N]
    ad = np.asarray(inputs["current_ad"], np.float32)        # [B, 8]
    B, N, nf = gn.shape

    flat = gn.reshape(-1, nf)                                # [B*N, nf]
    n_stat_cols = flat.shape[0] * nf // P
    stats = flat.reshape(16, -1, nf).transpose(0, 2, 1).reshape(P, n_stat_cols)

    nodesT = np.zeros((nf, n_cols), np.float32)
    nodesT[:, :N] = gn[b][order].T                           # rank order

    maskF = np.zeros((P, n_lg), np.float32)
    maskOff = np.full((P, n_lg), np.float32(NEG_INF))
    mrank = np.zeros(n_cols, bool)
    mrank[:N] = mask[b][order]
    mF = mrank.reshape(n_lg, P).T
    maskF[mF] = 1.0
    maskOff[mF] = 0.0

    sel = np.zeros((P, nf), np.float32)
    sel[np.arange(P), np.arange(P) % nf] = 1.0

    g = lambda k: np.ascontiguousarray(np.asarray(inputs[k], np.float32))
    col = lambda k: np.ascontiguousarray(
        np.asarray(inputs[k], np.float32).reshape(-1, 1))

    m = {
        "stats": np.ascontiguousarray(stats),
        "nodesT": np.ascontiguousarray(nodesT),
        "adT_all": np.ascontiguousarray(ad.T),
        "adT_own": np.ascontiguousarray(ad[b].reshape(-1, 1)),
        "idx": _wrap_idx(st["slot_srcs"][b]),
        "S": st["S"],
        "maskF": maskF, "maskOff": maskOff,
        "W_in": g("W_in"), "b_in": col("b_in"),
        "W_m1": g("W_m1"), "W_s1": g("W_s1"), "b1": col("b1"),
        "W_m2": g("W_m2"), "W_s2": g("W_s2"), "b2": col("b2"),
        "W_ad": g("W_ad"), "b_ad": col("b_ad"),
        "W_pt": g("W_p")[:D], "W_pb": g("W_p")[D:], "b_p": col("b_p"),
        "w_out": col("w_out"),
        "W_vt": g("W_v")[:D], "W_vb": g("W_v")[D:], "b_v": col("b_v"),
        "w_vo": col("w_vo"),
        "ident": np.eye(P, dtype=np.float32),
        "SEL": sel,
        "one11": np.ones((1, 1), np.float32),
        "ones128": np.ones((1, P), np.float32),
    }
    return m


_CACHE = {}


def _get_program(edge_links, iters):
    key = (hashlib.md5(np.asarray(edge_links).tobytes()).hexdigest(), iters)
    if key not in _CACHE:
        st = _prep_structure(np.asarray(edge_links), N_NODES)
        flat_sz = BATCH * N_NODES * NODE_F
        nc = build_program(st, N_NODES, NODE_F, AD_F, D, iters=iters,
                           n_stat_cols=flat_sz // P, n_ad=BATCH)
        _CACHE[key] = (st, nc)
    return _CACHE[key]


def kernel(graph_nodes, graph_edge_links, mask, current_ad, **weights):
    inputs = dict(graph_nodes=graph_nodes, graph_edge_links=graph_edge_links,
                  mask=mask, current_ad=current_ad, **weights)
    edge_links = np.asarray(graph_edge_links)
    st, nc = _get_program(edge_links, iters=int(os.environ.get("K_ITERS", 1)))

    in_maps = []
    for c in range(8):
        b = c // 2
        in_maps.append(_pack_inputs(inputs, st, b))

    res = run_bass_kernel_spmd(nc, in_maps, core_ids=list(range(8)))

    n_cols = st["n_cols"]
    log_probs = np.zeros((BATCH, N_NODES), np.float32)
    value = np.zeros((BATCH,), np.float32)
    for b in range(BATCH):
        r = res.results[2 * b]
        lp_rank = r["out_lp"].T.reshape(-1)[:N_NODES]     # rank-major
        order = st["orders"][b]
        log_probs[b][order] = lp_rank
        value[b] = r["out_val"][0, 0]
    return log_probs, value
